# revision 15
# baseline (speedup 1.0000x reference)
"""ARD kernel matrix on 8 TRN2 NeuronCores.

k(x, y) = exp(-0.5 * sum_d (x_d - y_d)^2 / bw_d),  bw = exp(log_bw)

Sharding: 4x2 grid over the [4096, 4096] output. Core c = (mg, ng) computes
the [1024, 2048] tile for x rows [mg*1024, +1024) and y rows [ng*2048, +2048).

Per-core device program (all engines overlapped via Tile):
  - s = exp(-0.5 * lbw) on ACT, per-partition scale of the D-major (transposed)
    x/y operands on DVE.
  - squared row norms via ones-vector matmuls on PE (reduction over the
    partition/contraction dim).
  - cross = xw^T yw via bf16 matmuls, fp32 PSUM accumulation; an augmented
    K=1 matmul adds -0.5*y2[j] into the same PSUM tile.
  - one ACT pass: out = exp(psum + bias) with per-partition bias -0.5*x2[i],
    written directly as bf16 and DMA'd out.
"""

import sys

import numpy as np

if "/opt/trn_rl_repo" not in sys.path:
    sys.path.insert(0, "/opt/trn_rl_repo")

import ml_dtypes

N, M, D = 4096, 4096, 256
MG, NG = 4, 2  # core grid: MG x-row groups x NG y-row groups
NL, ML = N // MG, M // NG  # per-core output tile: [1024, 2048]
KC = D // 128  # contraction chunks of 128
N_CORES = 8

_CACHE = {}
LAST_RESULT = None  # BassKernelResults of the most recent run (for profiling)


def _ensure_profile_hook():
    """Register the axon NTFF profile hook if the image's antenv lacks it.

    Only affects runs with BASS_TRACE=1; without it run_bass_kernel_spmd
    never consults the hook. Failures degrade to no-profile silently.
    """
    try:
        import contextlib
        import ctypes
        import types

        try:
            from antenv.axon_hooks import get_axon_ntff_profile_hook  # noqa: F401

            return  # real module present
        except ImportError:
            pass

        so_path = "/opt/axon/libaxon_pjrt.so"
        lib = ctypes.CDLL(so_path)
        if not hasattr(lib, "axon_start_nrt_profile"):
            return
        lib.axon_start_nrt_profile.argtypes = [
            ctypes.POINTER(ctypes.c_int64),
            ctypes.c_size_t,
        ]
        lib.axon_start_nrt_profile.restype = ctypes.c_int64
        lib.axon_stop_nrt_profile.argtypes = [ctypes.c_char_p]
        lib.axon_stop_nrt_profile.restype = ctypes.c_int64

        @contextlib.contextmanager
        def _hook(output_dir, device_ids):
            import jax

            jax.devices()
            if device_ids:
                ids = (ctypes.c_int64 * len(device_ids))(*device_ids)
                rc = lib.axon_start_nrt_profile(ids, len(device_ids))
            else:
                rc = lib.axon_start_nrt_profile(None, 0)
            if rc != 0:
                raise RuntimeError(f"axon_start_nrt_profile rc={rc}")
            try:
                yield
            finally:
                n = lib.axon_stop_nrt_profile(str(output_dir).encode())
                print(f"profile: {n} file(s) written to {output_dir}", file=sys.stderr)

        mod = types.ModuleType("antenv.axon_hooks")
        mod.get_axon_ntff_profile_hook = lambda: _hook
        mod.set_axon_ntff_profile_hook = lambda h: None
        sys.modules["antenv.axon_hooks"] = mod

        # artifact upload needs bucket creds this container may not have
        from concourse import bass_utils as _bu

        _bu.upload_artifacts = lambda tmpdir: tmpdir
    except Exception as e:  # pragma: no cover - profiling is best-effort
        print(f"profile hook setup failed: {e}", file=sys.stderr)


def _build_nc():
    from contextlib import ExitStack

    import concourse.tile as tile
    from concourse import bacc, mybir

    dt = mybir.dt
    FP32 = dt.float32
    BF16 = dt.bfloat16
    Act = mybir.ActivationFunctionType

    nc = bacc.Bacc()
    xt_d = nc.declare_dram_parameter("xt", [D, NL], BF16, isOutput=False)
    yt_d = nc.declare_dram_parameter("yt", [D, ML], BF16, isOutput=False)
    lbw_d = nc.declare_dram_parameter("lbw", [128, KC], FP32, isOutput=False)
    out_d = nc.declare_dram_parameter("out", [NL, ML], BF16, isOutput=True)

    n_mt = NL // 128  # 8 output row tiles
    n_ns = ML // 512  # 4 psum-bank columns per output row tile

    NSW = 512  # matmul moving free-dim (one PSUM bank)
    HW = 1024  # output half-tile width (2 PSUM banks)
    n_ht = ML // HW  # 2 half tiles per m row
    n_hs = HW // NSW  # 2 matmul spans per half tile

    with tile.TileContext(nc) as tc, ExitStack() as ctx:
        cpool = ctx.enter_context(tc.tile_pool(name="const", bufs=1))
        work = ctx.enter_context(tc.tile_pool(name="work", bufs=2))
        outp = ctx.enter_context(tc.tile_pool(name="outp", bufs=4))
        psum = ctx.enter_context(tc.tile_pool(name="psum", bufs=3, space="PSUM"))
        prep = ctx.enter_context(tc.tile_pool(name="prep", bufs=2, space="PSUM"))

        # out[i,j] = exp(cross_w[i,j] - 0.5*x2[i] - 0.5*y2[j]) in ONE ACT pass:
        #   cross_w = (s2 x)^T y accumulated in PSUM (bf16 matmuls)
        #   -0.5*y2[j] added into PSUM by an augmented K=1 matmul
        #   -0.5*x2[i] applied as the ACT per-partition bias

        # --- s2 = exp(-lbw), [128, KC] ---
        lbw_sb = cpool.tile([128, KC], FP32)
        nc.sync.dma_start(lbw_sb[:], lbw_d[:])
        s2_f = cpool.tile([128, KC], FP32)
        nc.scalar.activation(s2_f[:], lbw_sb[:], Act.Exp, scale=-1.0)
        s2_b = cpool.tile([128, KC], BF16)
        nc.vector.tensor_copy(s2_b[:], s2_f[:])

        ones_row = cpool.tile([1, 128], BF16)
        nc.vector.memset(ones_row[:], 1.0)

        # --- y loads first: the y2 -> negy2 chain gates the aug matmuls ---
        yraw = []
        for k in range(KC):
            yr = cpool.tile([128, ML], BF16, tag=f"yraw{k}")
            nc.sync.dma_start(yr[:], yt_d[128 * k : 128 * (k + 1), :])
            yraw.append(yr)
        xraw = []
        for k in range(KC):
            xr = cpool.tile([128, NL], BF16, tag=f"xraw{k}")
            nc.sync.dma_start(xr[:], xt_d[128 * k : 128 * (k + 1), :])
            xraw.append(xr)

        # --- y2/negy2 per 512-wide chunk: square (DVE), reduce (PE), scale-copy (DVE) ---
        negy2 = cpool.tile([1, ML], BF16)
        ysq = []
        for k in range(KC):
            ysq_k = work.tile([128, ML], BF16, tag=f"ysq{k}")
            ysq.append(ysq_k)
        for ns in range(n_ns):
            sl = slice(512 * ns, 512 * (ns + 1))
            for k in range(KC):
                nc.vector.tensor_mul(ysq[k][:, sl], yraw[k][:, sl], yraw[k][:, sl])
            py = prep.tile([128, 512], mybir.dt.float32, tag="prep")
            for k in range(KC):
                nc.tensor.matmul(
                    py[0:1, :],
                    lhsT=s2_b[:, k : k + 1],
                    rhs=ysq[k][:, sl],
                    start=(k == 0),
                    stop=(k == KC - 1),
                )
            nc.vector.tensor_scalar_mul(negy2[0:1, sl], py[0:1, :], -0.5)

        # --- x side: scaled operand + x2 -> negx2 bias ---
        xw2 = []
        xsq = []
        for k in range(KC):
            xw2_k = cpool.tile([128, NL], BF16, tag=f"xw2{k}")
            nc.vector.tensor_scalar_mul(xw2_k[:], xraw[k][:], s2_f[:, k : k + 1])
            xsq_k = work.tile([128, NL], BF16, tag=f"xsq{k}")
            nc.vector.tensor_mul(xsq_k[:], xraw[k][:], xraw[k][:])
            xw2.append(xw2_k)
            xsq.append(xsq_k)

        px = prep.tile([128, 512], mybir.dt.float32, tag="prep")
        for m in range(n_mt):
            for k in range(KC):
                nc.tensor.matmul(
                    px[:, m : m + 1],
                    lhsT=xsq[k][:, 128 * m : 128 * (m + 1)],
                    rhs=s2_b[:, k : k + 1],
                    start=(k == 0),
                    stop=(k == KC - 1),
                )
        negx2 = cpool.tile([128, n_mt], FP32)
        nc.vector.tensor_scalar_mul(negx2[:], px[:, 0:n_mt], -0.5)

        # --- main loop over [128, HW] half tiles ---
        for m in range(n_mt):
            for h in range(n_ht):
                ps = psum.tile([128, HW], mybir.dt.float32, tag="ps")
                for s in range(n_hs):
                    c = ps[:, NSW * s : NSW * (s + 1)]
                    off = HW * h + NSW * s
                    for k in range(KC):
                        nc.tensor.matmul(
                            c,
                            lhsT=xw2[k][:, 128 * m : 128 * (m + 1)],
                            rhs=yraw[k][:, off : off + NSW],
                            start=(k == 0),
                            stop=False,
                        )
                    nc.tensor.matmul(
                        c,
                        lhsT=ones_row[:],
                        rhs=negy2[0:1, off : off + NSW],
                        start=False,
                        stop=True,
                    )
                ob = outp.tile([128, HW], BF16, tag="ob")
                nc.scalar.activation(
                    ob[:], ps[:], Act.Exp, bias=negx2[:, m : m + 1], scale=1.0
                )
                nc.sync.dma_start(
                    out_d[128 * m : 128 * (m + 1), HW * h : HW * (h + 1)], ob[:]
                )

    nc.finalize()
    return nc


def _get_nc():
    if "nc" not in _CACHE:
        _CACHE["nc"] = _build_nc()
    return _CACHE["nc"]


def kernel(x, y, log_band_width):
    global LAST_RESULT
    _ensure_profile_hook()
    from concourse.bass_utils import run_bass_kernel_spmd

    nc = _get_nc()

    xtb = np.ascontiguousarray(x.astype(ml_dtypes.bfloat16).T)  # [D, N]
    ytb = np.ascontiguousarray(y.astype(ml_dtypes.bfloat16).T)  # [D, M]
    # lbw_t[p, k] = lbw[128k + p] so column k scales contraction chunk k
    lbw_t = np.ascontiguousarray(
        log_band_width.astype(np.float32).reshape(KC, 128).T
    )

    in_maps = []
    for c in range(N_CORES):
        mg, ng = divmod(c, NG)
        in_maps.append(
            {
                "xt": np.ascontiguousarray(xtb[:, mg * NL : (mg + 1) * NL]),
                "yt": np.ascontiguousarray(ytb[:, ng * ML : (ng + 1) * ML]),
                "lbw": lbw_t,
            }
        )

    res = run_bass_kernel_spmd(nc, in_maps, core_ids=list(range(N_CORES)))
    LAST_RESULT = res

    outs = [np.asarray(res.results[c]["out"]) for c in range(N_CORES)]
    rows = [
        np.concatenate([outs[mg * NG + ng] for ng in range(NG)], axis=1)
        for mg in range(MG)
    ]
    return np.concatenate(rows, axis=0).astype(np.float32)


# revision 16
# speedup vs baseline: 1.5482x; 1.5482x over previous
"""ARD kernel matrix on 8 TRN2 NeuronCores.

k(x, y) = exp(-0.5 * sum_d (x_d - y_d)^2 / bw_d),  bw = exp(log_bw)

Sharding: 4x2 grid over the [4096, 4096] output. Core c = (mg, ng) computes
the [1024, 2048] tile for x rows [mg*1024, +1024) and y rows [ng*2048, +2048).

Per-core device program (all engines overlapped via Tile):
  - s = exp(-0.5 * lbw) on ACT, per-partition scale of the D-major (transposed)
    x/y operands on DVE.
  - squared row norms via ones-vector matmuls on PE (reduction over the
    partition/contraction dim).
  - cross = xw^T yw via bf16 matmuls, fp32 PSUM accumulation; an augmented
    K=1 matmul adds -0.5*y2[j] into the same PSUM tile.
  - one ACT pass: out = exp(psum + bias) with per-partition bias -0.5*x2[i],
    written directly as bf16 and DMA'd out.
"""

import sys

import numpy as np

if "/opt/trn_rl_repo" not in sys.path:
    sys.path.insert(0, "/opt/trn_rl_repo")

import ml_dtypes

N, M, D = 4096, 4096, 256
MG, NG = 4, 2  # core grid: MG x-row groups x NG y-row groups
NL, ML = N // MG, M // NG  # per-core output tile: [1024, 2048]
KC = D // 128  # contraction chunks of 128
N_CORES = 8

_CACHE = {}
LAST_RESULT = None  # BassKernelResults of the most recent run (for profiling)


def _ensure_profile_hook():
    """Register the axon NTFF profile hook if the image's antenv lacks it.

    Only affects runs with BASS_TRACE=1; without it run_bass_kernel_spmd
    never consults the hook. Failures degrade to no-profile silently.
    """
    try:
        import contextlib
        import ctypes
        import types

        try:
            from antenv.axon_hooks import get_axon_ntff_profile_hook  # noqa: F401

            return  # real module present
        except ImportError:
            pass

        so_path = "/opt/axon/libaxon_pjrt.so"
        lib = ctypes.CDLL(so_path)
        if not hasattr(lib, "axon_start_nrt_profile"):
            return
        lib.axon_start_nrt_profile.argtypes = [
            ctypes.POINTER(ctypes.c_int64),
            ctypes.c_size_t,
        ]
        lib.axon_start_nrt_profile.restype = ctypes.c_int64
        lib.axon_stop_nrt_profile.argtypes = [ctypes.c_char_p]
        lib.axon_stop_nrt_profile.restype = ctypes.c_int64

        @contextlib.contextmanager
        def _hook(output_dir, device_ids):
            import jax

            jax.devices()
            if device_ids:
                ids = (ctypes.c_int64 * len(device_ids))(*device_ids)
                rc = lib.axon_start_nrt_profile(ids, len(device_ids))
            else:
                rc = lib.axon_start_nrt_profile(None, 0)
            if rc != 0:
                raise RuntimeError(f"axon_start_nrt_profile rc={rc}")
            try:
                yield
            finally:
                n = lib.axon_stop_nrt_profile(str(output_dir).encode())
                print(f"profile: {n} file(s) written to {output_dir}", file=sys.stderr)

        mod = types.ModuleType("antenv.axon_hooks")
        mod.get_axon_ntff_profile_hook = lambda: _hook
        mod.set_axon_ntff_profile_hook = lambda h: None
        sys.modules["antenv.axon_hooks"] = mod

        # artifact upload needs bucket creds this container may not have
        from concourse import bass_utils as _bu

        _bu.upload_artifacts = lambda tmpdir: tmpdir
    except Exception as e:  # pragma: no cover - profiling is best-effort
        print(f"profile hook setup failed: {e}", file=sys.stderr)


def _build_nc():
    from contextlib import ExitStack

    import concourse.tile as tile
    from concourse import bacc, mybir

    dt = mybir.dt
    FP32 = dt.float32
    BF16 = dt.bfloat16
    Act = mybir.ActivationFunctionType

    nc = bacc.Bacc()
    xt_d = nc.declare_dram_parameter("xt", [D, NL], BF16, isOutput=False)
    yt_d = nc.declare_dram_parameter("yt", [D, ML], BF16, isOutput=False)
    lbw_d = nc.declare_dram_parameter("lbw", [128, KC], FP32, isOutput=False)
    out_d = nc.declare_dram_parameter("out", [NL, ML], BF16, isOutput=True)

    n_mt = NL // 128  # 8 output row tiles
    n_ns = ML // 512  # 4 psum-bank columns per output row tile

    NSW = 512  # matmul moving free-dim (one PSUM bank)
    HW = 1024  # output half-tile width (2 PSUM banks)
    n_ht = ML // HW  # 2 half tiles per m row
    n_hs = HW // NSW  # 2 matmul spans per half tile

    with tile.TileContext(nc) as tc, ExitStack() as ctx:
        cpool = ctx.enter_context(tc.tile_pool(name="const", bufs=1))
        work = ctx.enter_context(tc.tile_pool(name="work", bufs=2))
        outp = ctx.enter_context(tc.tile_pool(name="outp", bufs=4))
        psum = ctx.enter_context(tc.tile_pool(name="psum", bufs=3, space="PSUM"))
        prep = ctx.enter_context(tc.tile_pool(name="prep", bufs=2, space="PSUM"))

        # out[i,j] = exp(cross_w[i,j] - 0.5*x2[i] - 0.5*y2[j]) in ONE ACT pass:
        #   cross_w = (s2 x)^T y accumulated in PSUM (bf16 matmuls)
        #   -0.5*y2[j] added into PSUM by an augmented K=1 matmul
        #   -0.5*x2[i] applied as the ACT per-partition bias

        # --- s2 = exp(-lbw), [128, KC] ---
        lbw_sb = cpool.tile([128, KC], FP32)
        nc.sync.dma_start(lbw_sb[:], lbw_d[:])
        s2_f = cpool.tile([128, KC], FP32)
        nc.scalar.activation(s2_f[:], lbw_sb[:], Act.Exp, scale=-1.0)
        s2_b = cpool.tile([128, KC], BF16)
        nc.vector.tensor_copy(s2_b[:], s2_f[:])

        # Full-K augmented-matmul operands: row 0 carries the data, rows
        # 1-127 are zero. K=1 matmuls starve the PE activity monitor (HAM)
        # and keep the array at the cold 1.2 GHz clock; K=128 keeps it warm.
        aug_lhsT = cpool.tile([128, 128], BF16)
        nc.vector.memset(aug_lhsT[:], 0.0)
        nc.vector.memset(aug_lhsT[0:1, :], 1.0)
        negy2_pad = cpool.tile([128, ML], BF16)
        nc.vector.memset(negy2_pad[:], 0.0)

        # --- y loads first: the y2 -> negy2 chain gates the aug matmuls ---
        yraw = []
        for k in range(KC):
            yr = cpool.tile([128, ML], BF16, tag=f"yraw{k}")
            nc.sync.dma_start(yr[:], yt_d[128 * k : 128 * (k + 1), :])
            yraw.append(yr)
        xraw = []
        for k in range(KC):
            xr = cpool.tile([128, NL], BF16, tag=f"xraw{k}")
            nc.sync.dma_start(xr[:], xt_d[128 * k : 128 * (k + 1), :])
            xraw.append(xr)

        # --- y2/negy2 per 512-wide chunk: square (DVE), reduce (PE), scale-copy (DVE) ---
        ysq = []
        for k in range(KC):
            ysq_k = work.tile([128, ML], BF16, tag=f"ysq{k}")
            ysq.append(ysq_k)
        for ns in range(n_ns):
            sl = slice(512 * ns, 512 * (ns + 1))
            for k in range(KC):
                nc.vector.tensor_mul(ysq[k][:, sl], yraw[k][:, sl], yraw[k][:, sl])
            py = prep.tile([128, 512], mybir.dt.float32, tag="prep")
            for k in range(KC):
                nc.tensor.matmul(
                    py[0:1, :],
                    lhsT=s2_b[:, k : k + 1],
                    rhs=ysq[k][:, sl],
                    start=(k == 0),
                    stop=(k == KC - 1),
                )
            nc.vector.tensor_scalar_mul(negy2_pad[0:1, sl], py[0:1, :], -0.5)

        # --- x side: scaled operand + x2 -> negx2 bias ---
        xw2 = []
        xsq = []
        for k in range(KC):
            xw2_k = cpool.tile([128, NL], BF16, tag=f"xw2{k}")
            nc.vector.tensor_scalar_mul(xw2_k[:], xraw[k][:], s2_f[:, k : k + 1])
            xsq_k = work.tile([128, NL], BF16, tag=f"xsq{k}")
            nc.vector.tensor_mul(xsq_k[:], xraw[k][:], xraw[k][:])
            xw2.append(xw2_k)
            xsq.append(xsq_k)

        px = prep.tile([128, 512], mybir.dt.float32, tag="prep")
        for m in range(n_mt):
            for k in range(KC):
                nc.tensor.matmul(
                    px[:, m : m + 1],
                    lhsT=xsq[k][:, 128 * m : 128 * (m + 1)],
                    rhs=s2_b[:, k : k + 1],
                    start=(k == 0),
                    stop=(k == KC - 1),
                )
        negx2 = cpool.tile([128, n_mt], FP32)
        nc.vector.tensor_scalar_mul(negx2[:], px[:, 0:n_mt], -0.5)

        # --- main loop over [128, HW] half tiles ---
        for m in range(n_mt):
            for h in range(n_ht):
                ps = psum.tile([128, HW], mybir.dt.float32, tag="ps")
                for s in range(n_hs):
                    c = ps[:, NSW * s : NSW * (s + 1)]
                    off = HW * h + NSW * s
                    for k in range(KC):
                        nc.tensor.matmul(
                            c,
                            lhsT=xw2[k][:, 128 * m : 128 * (m + 1)],
                            rhs=yraw[k][:, off : off + NSW],
                            start=(k == 0),
                            stop=False,
                        )
                    nc.tensor.matmul(
                        c,
                        lhsT=aug_lhsT[:],
                        rhs=negy2_pad[:, off : off + NSW],
                        start=False,
                        stop=True,
                    )
                ob = outp.tile([128, HW], BF16, tag="ob")
                nc.scalar.activation(
                    ob[:], ps[:], Act.Exp, bias=negx2[:, m : m + 1], scale=1.0
                )
                nc.sync.dma_start(
                    out_d[128 * m : 128 * (m + 1), HW * h : HW * (h + 1)], ob[:]
                )

    nc.finalize()
    return nc


def _get_nc():
    if "nc" not in _CACHE:
        _CACHE["nc"] = _build_nc()
    return _CACHE["nc"]


def kernel(x, y, log_band_width):
    global LAST_RESULT
    _ensure_profile_hook()
    from concourse.bass_utils import run_bass_kernel_spmd

    nc = _get_nc()

    xtb = np.ascontiguousarray(x.astype(ml_dtypes.bfloat16).T)  # [D, N]
    ytb = np.ascontiguousarray(y.astype(ml_dtypes.bfloat16).T)  # [D, M]
    # lbw_t[p, k] = lbw[128k + p] so column k scales contraction chunk k
    lbw_t = np.ascontiguousarray(
        log_band_width.astype(np.float32).reshape(KC, 128).T
    )

    in_maps = []
    for c in range(N_CORES):
        mg, ng = divmod(c, NG)
        in_maps.append(
            {
                "xt": np.ascontiguousarray(xtb[:, mg * NL : (mg + 1) * NL]),
                "yt": np.ascontiguousarray(ytb[:, ng * ML : (ng + 1) * ML]),
                "lbw": lbw_t,
            }
        )

    res = run_bass_kernel_spmd(nc, in_maps, core_ids=list(range(N_CORES)))
    LAST_RESULT = res

    outs = [np.asarray(res.results[c]["out"]) for c in range(N_CORES)]
    rows = [
        np.concatenate([outs[mg * NG + ng] for ng in range(NG)], axis=1)
        for mg in range(MG)
    ]
    return np.concatenate(rows, axis=0).astype(np.float32)


# revision 18
# speedup vs baseline: 1.5897x; 1.0268x over previous
"""ARD kernel matrix on 8 TRN2 NeuronCores.

k(x, y) = exp(-0.5 * sum_d (x_d - y_d)^2 / bw_d),  bw = exp(log_bw)

Sharding: 4x2 grid over the [4096, 4096] output. Core c = (mg, ng) computes
the [1024, 2048] tile for x rows [mg*1024, +1024) and y rows [ng*2048, +2048).

Per-core device program (all engines overlapped via Tile):
  - s = exp(-0.5 * lbw) on ACT, per-partition scale of the D-major (transposed)
    x/y operands on DVE.
  - squared row norms via ones-vector matmuls on PE (reduction over the
    partition/contraction dim).
  - cross = xw^T yw via bf16 matmuls, fp32 PSUM accumulation; an augmented
    K=1 matmul adds -0.5*y2[j] into the same PSUM tile.
  - one ACT pass: out = exp(psum + bias) with per-partition bias -0.5*x2[i],
    written directly as bf16 and DMA'd out.
"""

import sys

import numpy as np

if "/opt/trn_rl_repo" not in sys.path:
    sys.path.insert(0, "/opt/trn_rl_repo")

import ml_dtypes

N, M, D = 4096, 4096, 256
MG, NG = 4, 2  # core grid: MG x-row groups x NG y-row groups
NL, ML = N // MG, M // NG  # per-core output tile: [1024, 2048]
KC = D // 128  # contraction chunks of 128
N_CORES = 8

_CACHE = {}
LAST_RESULT = None  # BassKernelResults of the most recent run (for profiling)


def _ensure_profile_hook():
    """Register the axon NTFF profile hook if the image's antenv lacks it.

    Only affects runs with BASS_TRACE=1; without it run_bass_kernel_spmd
    never consults the hook. Failures degrade to no-profile silently.
    """
    try:
        import contextlib
        import ctypes
        import types

        try:
            from antenv.axon_hooks import get_axon_ntff_profile_hook  # noqa: F401

            return  # real module present
        except ImportError:
            pass

        so_path = "/opt/axon/libaxon_pjrt.so"
        lib = ctypes.CDLL(so_path)
        if not hasattr(lib, "axon_start_nrt_profile"):
            return
        lib.axon_start_nrt_profile.argtypes = [
            ctypes.POINTER(ctypes.c_int64),
            ctypes.c_size_t,
        ]
        lib.axon_start_nrt_profile.restype = ctypes.c_int64
        lib.axon_stop_nrt_profile.argtypes = [ctypes.c_char_p]
        lib.axon_stop_nrt_profile.restype = ctypes.c_int64

        @contextlib.contextmanager
        def _hook(output_dir, device_ids):
            import jax

            jax.devices()
            if device_ids:
                ids = (ctypes.c_int64 * len(device_ids))(*device_ids)
                rc = lib.axon_start_nrt_profile(ids, len(device_ids))
            else:
                rc = lib.axon_start_nrt_profile(None, 0)
            if rc != 0:
                raise RuntimeError(f"axon_start_nrt_profile rc={rc}")
            try:
                yield
            finally:
                n = lib.axon_stop_nrt_profile(str(output_dir).encode())
                print(f"profile: {n} file(s) written to {output_dir}", file=sys.stderr)

        mod = types.ModuleType("antenv.axon_hooks")
        mod.get_axon_ntff_profile_hook = lambda: _hook
        mod.set_axon_ntff_profile_hook = lambda h: None
        sys.modules["antenv.axon_hooks"] = mod

        # artifact upload needs bucket creds this container may not have
        from concourse import bass_utils as _bu

        _bu.upload_artifacts = lambda tmpdir: tmpdir
    except Exception as e:  # pragma: no cover - profiling is best-effort
        print(f"profile hook setup failed: {e}", file=sys.stderr)


def _build_nc():
    from contextlib import ExitStack

    import concourse.tile as tile
    from concourse import bacc, mybir

    dt = mybir.dt
    FP32 = dt.float32
    BF16 = dt.bfloat16
    Act = mybir.ActivationFunctionType

    nc = bacc.Bacc()
    xt_d = nc.declare_dram_parameter("xt", [D, NL], BF16, isOutput=False)
    yt_d = nc.declare_dram_parameter("yt", [D, ML], BF16, isOutput=False)
    lbw_d = nc.declare_dram_parameter("lbw", [128, KC], FP32, isOutput=False)
    out_d = nc.declare_dram_parameter("out", [NL, ML], BF16, isOutput=True)

    n_mt = NL // 128  # 8 output row tiles
    n_ns = ML // 512  # 4 psum-bank columns per output row tile

    NSW = 512  # matmul moving free-dim (one PSUM bank)
    HW = 1024  # output half-tile width (2 PSUM banks)
    n_ht = ML // HW  # 2 half tiles per m row
    n_sp = ML // NSW  # 4 matmul spans per m row

    with tile.TileContext(nc) as tc, ExitStack() as ctx:
        cpool = ctx.enter_context(tc.tile_pool(name="const", bufs=1))
        work = ctx.enter_context(tc.tile_pool(name="work", bufs=2))
        outp = ctx.enter_context(tc.tile_pool(name="outp", bufs=4))
        psum = ctx.enter_context(tc.tile_pool(name="psum", bufs=3, space="PSUM"))
        prep = ctx.enter_context(tc.tile_pool(name="prep", bufs=2, space="PSUM"))

        # out[i,j] = exp(cross_w[i,j] - 0.5*x2[i] - 0.5*y2[j]) in ONE ACT pass:
        #   cross_w = (s2 x)^T y accumulated in PSUM (bf16 matmuls, k-outer so
        #   one weight load covers all 4 moving spans)
        #   -0.5*y2[j] added into PSUM by a zero-padded K=128 matmul (a K=1
        #   matmul starves the PE activity monitor and drops the clock)
        #   -0.5*x2[i] applied as the ACT per-partition bias

        # --- s2 = exp(-lbw), [128, KC] ---
        lbw_sb = cpool.tile([128, KC], FP32)
        nc.sync.dma_start(lbw_sb[:], lbw_d[:])
        s2_f = cpool.tile([128, KC], FP32)
        nc.scalar.activation(s2_f[:], lbw_sb[:], Act.Exp, scale=-1.0)
        s2_b = cpool.tile([128, KC], BF16)
        nc.vector.tensor_copy(s2_b[:], s2_f[:])

        # --- loads: x first (it gates the mains + bias chain) ---
        xraw = []
        for k in range(KC):
            xr = cpool.tile([128, NL], BF16, tag=f"xraw{k}")
            nc.sync.dma_start(xr[:], xt_d[128 * k : 128 * (k + 1), :])
            xraw.append(xr)
        yraw = []
        for k in range(KC):
            yr = cpool.tile([128, ML], BF16, tag=f"yraw{k}")
            nc.sync.dma_start(yr[:], yt_d[128 * k : 128 * (k + 1), :])
            yraw.append(yr)

        # --- x side: scaled operand, squares, x2, bias ---
        xw2 = []
        xsq = []
        for k in range(KC):
            xw2_k = cpool.tile([128, NL], BF16, tag=f"xw2{k}")
            nc.vector.tensor_scalar_mul(xw2_k[:], xraw[k][:], s2_f[:, k : k + 1])
            xsq_k = work.tile([128, NL], BF16, tag=f"xsq{k}")
            nc.vector.tensor_mul(xsq_k[:], xraw[k][:], xraw[k][:])
            xw2.append(xw2_k)
            xsq.append(xsq_k)

        px = prep.tile([128, 512], mybir.dt.float32, tag="prep")
        for m in range(n_mt):
            for k in range(KC):
                nc.tensor.matmul(
                    px[:, m : m + 1],
                    lhsT=xsq[k][:, 128 * m : 128 * (m + 1)],
                    rhs=s2_b[:, k : k + 1],
                    start=(k == 0),
                    stop=(k == KC - 1),
                )
        # aug operands (memsets early, cheap)
        aug_lhsT = cpool.tile([128, 128], BF16)
        nc.vector.memset(aug_lhsT[:], 0.0)
        nc.vector.memset(aug_lhsT[0:1, :], 1.0)
        negy2_pad = cpool.tile([128, ML], BF16)
        nc.vector.memset(negy2_pad[:], 0.0)

        # --- first m row's cross matmuls keep PE busy during y prep ---
        def main_mms(pss, m):
            for k in range(KC):
                for s in range(n_sp):
                    c = pss[s // 2][:, NSW * (s % 2) : NSW * (s % 2 + 1)]
                    nc.tensor.matmul(
                        c,
                        lhsT=xw2[k][:, 128 * m : 128 * (m + 1)],
                        rhs=yraw[k][:, NSW * s : NSW * (s + 1)],
                        start=(k == 0),
                        stop=False,
                    )

        def finish_mt(pss, m):
            for s in range(n_sp):
                nc.tensor.matmul(
                    pss[s // 2][:, NSW * (s % 2) : NSW * (s % 2 + 1)],
                    lhsT=aug_lhsT[:],
                    rhs=negy2_pad[:, NSW * s : NSW * (s + 1)],
                    start=False,
                    stop=True,
                )
            for h in range(n_ht):
                ob = outp.tile([128, HW], BF16, tag="ob")
                nc.scalar.activation(
                    ob[:], pss[h][:], Act.Exp, bias=negx2[:, m : m + 1], scale=1.0
                )
                nc.sync.dma_start(
                    out_d[128 * m : 128 * (m + 1), HW * h : HW * (h + 1)], ob[:]
                )

        pss0 = [psum.tile([128, HW], mybir.dt.float32, tag="ps", name=f"ps0_{_h}") for _h in range(n_ht)]
        main_mms(pss0, 0)

        # --- y2/negy2 per 512-wide span: square (DVE), reduce (PE), copy (DVE) ---
        ysq = []
        for k in range(KC):
            ysq_k = work.tile([128, ML], BF16, tag=f"ysq{k}")
            ysq.append(ysq_k)
        for ns in range(n_sp):
            sl = slice(512 * ns, 512 * (ns + 1))
            for k in range(KC):
                nc.vector.tensor_mul(ysq[k][:, sl], yraw[k][:, sl], yraw[k][:, sl])
            py = prep.tile([128, 512], mybir.dt.float32, tag="prep")
            for k in range(KC):
                nc.tensor.matmul(
                    py[0:1, :],
                    lhsT=s2_b[:, k : k + 1],
                    rhs=ysq[k][:, sl],
                    start=(k == 0),
                    stop=(k == KC - 1),
                )
            nc.vector.tensor_scalar_mul(negy2_pad[0:1, sl], py[0:1, :], -0.5)

        negx2 = cpool.tile([128, n_mt], FP32)
        nc.vector.tensor_scalar_mul(negx2[:], px[:, 0:n_mt], -0.5)

        # --- finish m=0, then the rest ---
        finish_mt(pss0, 0)
        for m in range(1, n_mt):
            pss = [psum.tile([128, HW], mybir.dt.float32, tag="ps", name=f"ps{m}_{_h}") for _h in range(n_ht)]
            main_mms(pss, m)
            finish_mt(pss, m)

    nc.finalize()
    return nc


def _get_nc():
    if "nc" not in _CACHE:
        _CACHE["nc"] = _build_nc()
    return _CACHE["nc"]


def kernel(x, y, log_band_width):
    global LAST_RESULT
    _ensure_profile_hook()
    from concourse.bass_utils import run_bass_kernel_spmd

    nc = _get_nc()

    xtb = np.ascontiguousarray(x.astype(ml_dtypes.bfloat16).T)  # [D, N]
    ytb = np.ascontiguousarray(y.astype(ml_dtypes.bfloat16).T)  # [D, M]
    # lbw_t[p, k] = lbw[128k + p] so column k scales contraction chunk k
    lbw_t = np.ascontiguousarray(
        log_band_width.astype(np.float32).reshape(KC, 128).T
    )

    in_maps = []
    for c in range(N_CORES):
        mg, ng = divmod(c, NG)
        in_maps.append(
            {
                "xt": np.ascontiguousarray(xtb[:, mg * NL : (mg + 1) * NL]),
                "yt": np.ascontiguousarray(ytb[:, ng * ML : (ng + 1) * ML]),
                "lbw": lbw_t,
            }
        )

    res = run_bass_kernel_spmd(nc, in_maps, core_ids=list(range(N_CORES)))
    LAST_RESULT = res

    outs = [np.asarray(res.results[c]["out"]) for c in range(N_CORES)]
    rows = [
        np.concatenate([outs[mg * NG + ng] for ng in range(NG)], axis=1)
        for mg in range(MG)
    ]
    return np.concatenate(rows, axis=0).astype(np.float32)


# revision 20
# speedup vs baseline: 1.8141x; 1.1412x over previous
"""ARD kernel matrix on 8 TRN2 NeuronCores.

k(x, y) = exp(-0.5 * sum_d (x_d - y_d)^2 / bw_d),  bw = exp(log_bw)

Sharding: 4x2 grid over the [4096, 4096] output. Core c = (mg, ng) computes
the [1024, 2048] tile for x rows [mg*1024, +1024) and y rows [ng*2048, +2048).

Per-core device program (all engines overlapped via Tile):
  - s = exp(-0.5 * lbw) on ACT, per-partition scale of the D-major (transposed)
    x/y operands on DVE.
  - squared row norms via ones-vector matmuls on PE (reduction over the
    partition/contraction dim).
  - cross = xw^T yw via bf16 matmuls, fp32 PSUM accumulation; an augmented
    K=1 matmul adds -0.5*y2[j] into the same PSUM tile.
  - one ACT pass: out = exp(psum + bias) with per-partition bias -0.5*x2[i],
    written directly as bf16 and DMA'd out.
"""

import sys

import numpy as np

if "/opt/trn_rl_repo" not in sys.path:
    sys.path.insert(0, "/opt/trn_rl_repo")

import ml_dtypes

N, M, D = 4096, 4096, 256
MG, NG = 4, 2  # core grid: MG x-row groups x NG y-row groups
NL, ML = N // MG, M // NG  # per-core output tile: [1024, 2048]
KC = D // 128  # contraction chunks of 128
N_CORES = 8

_CACHE = {}
LAST_RESULT = None  # BassKernelResults of the most recent run (for profiling)


def _ensure_profile_hook():
    """Register the axon NTFF profile hook if the image's antenv lacks it.

    Only affects runs with BASS_TRACE=1; without it run_bass_kernel_spmd
    never consults the hook. Failures degrade to no-profile silently.
    """
    try:
        import contextlib
        import ctypes
        import types

        try:
            from antenv.axon_hooks import get_axon_ntff_profile_hook  # noqa: F401

            return  # real module present
        except ImportError:
            pass

        so_path = "/opt/axon/libaxon_pjrt.so"
        lib = ctypes.CDLL(so_path)
        if not hasattr(lib, "axon_start_nrt_profile"):
            return
        lib.axon_start_nrt_profile.argtypes = [
            ctypes.POINTER(ctypes.c_int64),
            ctypes.c_size_t,
        ]
        lib.axon_start_nrt_profile.restype = ctypes.c_int64
        lib.axon_stop_nrt_profile.argtypes = [ctypes.c_char_p]
        lib.axon_stop_nrt_profile.restype = ctypes.c_int64

        @contextlib.contextmanager
        def _hook(output_dir, device_ids):
            import jax

            jax.devices()
            if device_ids:
                ids = (ctypes.c_int64 * len(device_ids))(*device_ids)
                rc = lib.axon_start_nrt_profile(ids, len(device_ids))
            else:
                rc = lib.axon_start_nrt_profile(None, 0)
            if rc != 0:
                raise RuntimeError(f"axon_start_nrt_profile rc={rc}")
            try:
                yield
            finally:
                n = lib.axon_stop_nrt_profile(str(output_dir).encode())
                print(f"profile: {n} file(s) written to {output_dir}", file=sys.stderr)

        mod = types.ModuleType("antenv.axon_hooks")
        mod.get_axon_ntff_profile_hook = lambda: _hook
        mod.set_axon_ntff_profile_hook = lambda h: None
        sys.modules["antenv.axon_hooks"] = mod

        # artifact upload needs bucket creds this container may not have
        from concourse import bass_utils as _bu

        _bu.upload_artifacts = lambda tmpdir: tmpdir
    except Exception as e:  # pragma: no cover - profiling is best-effort
        print(f"profile hook setup failed: {e}", file=sys.stderr)


def _build_nc():
    from contextlib import ExitStack

    import concourse.tile as tile
    from concourse import bacc, mybir

    dt = mybir.dt
    FP32 = dt.float32
    BF16 = dt.bfloat16
    Act = mybir.ActivationFunctionType

    nc = bacc.Bacc()
    xt_d = nc.declare_dram_parameter("xt", [D, NL], BF16, isOutput=False)
    yt_d = nc.declare_dram_parameter("yt", [D, ML], BF16, isOutput=False)
    lbw_d = nc.declare_dram_parameter("lbw", [128, KC], FP32, isOutput=False)
    out_d = nc.declare_dram_parameter("out", [NL, ML], BF16, isOutput=True)

    n_mt = NL // 128  # 8 output row tiles
    n_ns = ML // 512  # 4 psum-bank columns per output row tile

    NSW = 512  # matmul moving free-dim (one PSUM bank)
    HW = 1024  # output half-tile width (2 PSUM banks)
    n_ht = ML // HW  # 2 half tiles per m row
    n_sp = ML // NSW  # 4 matmul spans per m row

    with tile.TileContext(nc) as tc, ExitStack() as ctx:
        cpool = ctx.enter_context(tc.tile_pool(name="const", bufs=1))
        work = ctx.enter_context(tc.tile_pool(name="work", bufs=2))
        outp = ctx.enter_context(tc.tile_pool(name="outp", bufs=4))
        psum = ctx.enter_context(tc.tile_pool(name="psum", bufs=3, space="PSUM"))
        prep = ctx.enter_context(tc.tile_pool(name="prep", bufs=2, space="PSUM"))

        # out[i,j] = exp(cross_w[i,j] - 0.5*x2[i] - 0.5*y2[j]) in ONE ACT pass:
        #   cross_w = (s2 x)^T y accumulated in PSUM (bf16 matmuls, k-outer so
        #   one weight load covers all 4 moving spans)
        #   -0.5*y2[j] added into PSUM by a zero-padded K=128 matmul (a K=1
        #   matmul starves the PE activity monitor and drops the clock)
        #   -0.5*x2[i] applied as the ACT per-partition bias

        # --- s2 = exp(-lbw), [128, KC] ---
        lbw_sb = cpool.tile([128, KC], FP32)
        nc.sync.dma_start(lbw_sb[:], lbw_d[:])
        s2_f = cpool.tile([128, KC], FP32)
        nc.scalar.activation(s2_f[:], lbw_sb[:], Act.Exp, scale=-1.0)
        s2_b = cpool.tile([128, KC], BF16)
        nc.vector.tensor_copy(s2_b[:], s2_f[:])

        # --- loads: x first (it gates the mains + bias chain) ---
        xraw = []
        for k in range(KC):
            xr = cpool.tile([128, NL], BF16, tag=f"xraw{k}")
            nc.sync.dma_start(xr[:], xt_d[128 * k : 128 * (k + 1), :])
            xraw.append(xr)
        yraw = []
        for k in range(KC):
            yr = cpool.tile([128, ML], BF16, tag=f"yraw{k}")
            nc.sync.dma_start(yr[:], yt_d[128 * k : 128 * (k + 1), :])
            yraw.append(yr)

        # --- x side: scaled operand, squares, x2, bias ---
        xw2 = []
        xsq = []
        for k in range(KC):
            xw2_k = cpool.tile([128, NL], BF16, tag=f"xw2{k}")
            nc.vector.tensor_scalar_mul(xw2_k[:], xraw[k][:], s2_f[:, k : k + 1])
            xsq_k = work.tile([128, NL], BF16, tag=f"xsq{k}")
            nc.vector.tensor_mul(xsq_k[:], xraw[k][:], xraw[k][:])
            xw2.append(xw2_k)
            xsq.append(xsq_k)

        px = prep.tile([128, 512], mybir.dt.float32, tag="prep")
        for m in range(n_mt):
            for k in range(KC):
                nc.tensor.matmul(
                    px[:, m : m + 1],
                    lhsT=xsq[k][:, 128 * m : 128 * (m + 1)],
                    rhs=s2_b[:, k : k + 1],
                    start=(k == 0),
                    stop=(k == KC - 1),
                )
        negx2 = cpool.tile([128, n_mt], FP32)
        nc.vector.tensor_scalar_mul(negx2[:], px[:, 0:n_mt], -0.5)

        # aug operands (memsets early, cheap)
        aug_lhsT = cpool.tile([128, 128], BF16)
        nc.gpsimd.memset(aug_lhsT[:], 0.0)
        nc.gpsimd.memset(aug_lhsT[0:1, :], 1.0)
        negy2_pad = cpool.tile([128, ML], BF16)
        nc.gpsimd.memset(negy2_pad[:], 0.0)

        # --- first m row's cross matmuls keep PE busy during y prep ---
        def main_mms(pss, m):
            for k in range(KC):
                for s in range(n_sp):
                    c = pss[s // 2][:, NSW * (s % 2) : NSW * (s % 2 + 1)]
                    nc.tensor.matmul(
                        c,
                        lhsT=xw2[k][:, 128 * m : 128 * (m + 1)],
                        rhs=yraw[k][:, NSW * s : NSW * (s + 1)],
                        start=(k == 0),
                        stop=False,
                    )

        def finish_mt(pss, m):
            for s in range(n_sp):
                nc.tensor.matmul(
                    pss[s // 2][:, NSW * (s % 2) : NSW * (s % 2 + 1)],
                    lhsT=aug_lhsT[:],
                    rhs=negy2_pad[:, NSW * s : NSW * (s + 1)],
                    start=False,
                    stop=True,
                )
            for h in range(n_ht):
                ob = outp.tile([128, HW], BF16, tag="ob")
                nc.scalar.activation(
                    ob[:], pss[h][:], Act.Exp, bias=negx2[:, m : m + 1], scale=1.0
                )
                nc.sync.dma_start(
                    out_d[128 * m : 128 * (m + 1), HW * h : HW * (h + 1)], ob[:]
                )

        pss0 = [psum.tile([128, HW], mybir.dt.float32, tag="ps", name=f"ps0_{_h}") for _h in range(n_ht)]
        main_mms(pss0, 0)

        # --- y2/negy2 per 512-wide span: square (DVE), reduce (PE), copy (DVE) ---
        ysq = []
        for k in range(KC):
            ysq_k = work.tile([128, ML], BF16, tag=f"ysq{k}")
            ysq.append(ysq_k)
        for ns in range(n_sp):
            sl = slice(512 * ns, 512 * (ns + 1))
            for k in range(KC):
                nc.vector.tensor_mul(ysq[k][:, sl], yraw[k][:, sl], yraw[k][:, sl])
            py = prep.tile([128, 512], mybir.dt.float32, tag="prep")
            for k in range(KC):
                nc.tensor.matmul(
                    py[0:1, :],
                    lhsT=s2_b[:, k : k + 1],
                    rhs=ysq[k][:, sl],
                    start=(k == 0),
                    stop=(k == KC - 1),
                )
            nc.vector.tensor_scalar_mul(negy2_pad[0:1, sl], py[0:1, :], -0.5)

        # --- finish m=0, then the rest ---
        finish_mt(pss0, 0)
        for m in range(1, n_mt):
            pss = [psum.tile([128, HW], mybir.dt.float32, tag="ps", name=f"ps{m}_{_h}") for _h in range(n_ht)]
            main_mms(pss, m)
            finish_mt(pss, m)

    nc.finalize()
    return nc


def _get_nc():
    if "nc" not in _CACHE:
        _CACHE["nc"] = _build_nc()
    return _CACHE["nc"]


def kernel(x, y, log_band_width):
    global LAST_RESULT
    _ensure_profile_hook()
    from concourse.bass_utils import run_bass_kernel_spmd

    nc = _get_nc()

    xtb = np.ascontiguousarray(x.astype(ml_dtypes.bfloat16).T)  # [D, N]
    ytb = np.ascontiguousarray(y.astype(ml_dtypes.bfloat16).T)  # [D, M]
    # lbw_t[p, k] = lbw[128k + p] so column k scales contraction chunk k
    lbw_t = np.ascontiguousarray(
        log_band_width.astype(np.float32).reshape(KC, 128).T
    )

    in_maps = []
    for c in range(N_CORES):
        mg, ng = divmod(c, NG)
        in_maps.append(
            {
                "xt": np.ascontiguousarray(xtb[:, mg * NL : (mg + 1) * NL]),
                "yt": np.ascontiguousarray(ytb[:, ng * ML : (ng + 1) * ML]),
                "lbw": lbw_t,
            }
        )

    res = run_bass_kernel_spmd(nc, in_maps, core_ids=list(range(N_CORES)))
    LAST_RESULT = res

    outs = [np.asarray(res.results[c]["out"]) for c in range(N_CORES)]
    rows = [
        np.concatenate([outs[mg * NG + ng] for ng in range(NG)], axis=1)
        for mg in range(MG)
    ]
    return np.concatenate(rows, axis=0).astype(np.float32)
